# revision 13
# baseline (speedup 1.0000x reference)
"""Gemma2 sliding-window attention (B=1, S=4096, HID=3584, 16 Q heads / 8 KV heads,
HD=256, window 2047, tanh softcap 50) on 8 Trainium2 NeuronCores.

Sharding: tensor-parallel over heads. Core c owns Q heads (2c, 2c+1) and KV head c.
  - QKV projection computed transposed ([feature, token] layout) so Q/K land in the
    layout the scores matmul needs with zero on-device transposes. V is computed in
    [token, feature] layout for the PV matmul.
  - Scores are computed transposed ([k, q] tiles); softmax uses no max-subtraction
    (tanh softcap bounds scores to +-50 so exp cannot overflow); masking is
    multiplicative post-exp with 8 precomputed boundary masks. The softmax
    denominator is accumulated on the vector engine (f32) and collapsed across
    partitions with a gpsimd partition_all_reduce -- no M=1 matmuls.
  - Attention blocks are software-pipelined ACROSS (qb, h) block boundaries so the
    scalar-engine tanh/exp latency never stalls the PE at block edges.
  - Output projection is computed per 512-token block as a PARTIAL product against
    this core's 512 attention features (out.T layout: [3584 hid, 512 tok]), emitted
    interleaved with the next block's attention matmuls, then summed across cores
    with a per-block ReduceScatter (bf16). No AllGather, no full-attn DRAM round
    trip; only the last block's o-proj + RS are exposed at the tail.
Host side: weights are pre-transposed/pre-cast to bf16, RoPE cos/sin tables are
precomputed from position_ids; output stripes are reassembled on host.
"""

import sys

if "/opt/trn_rl_repo" not in sys.path:
    sys.path.insert(0, "/opt/trn_rl_repo")

import numpy as np
import ml_dtypes

import concourse.bass as bass
import concourse.bass_isa as bass_isa
import concourse.tile as tile
from concourse import bacc, mybir
from concourse.bass_utils import run_bass_kernel_spmd

# Problem constants (hardcoded per harness contract)
S = 4096
HID = 3584
NH, NKV, HD = 16, 8, 256
Q_SIZE = NH * HD          # 4096
SCALE = 256.0 ** -0.5     # 1/16
SOFTCAP = 50.0
WINDOW = 2048 - 1         # 2047
THETA = 10000.0

N_CORES = 8
QK_F = 2 * HD + HD        # 768 per-core transposed-qk features: [q_h0, q_h1, k]
HOUT = HID // N_CORES     # 448 output rows per core after ReduceScatter
KO = HID // 128           # 28 contraction subtiles for projections
TT = S // 512             # 8 token tiles of 512
HP = HID // 128           # 28 o-proj hid chunks
F32 = mybir.dt.float32
BF16 = mybir.dt.bfloat16

# Boundary-tile diagonal offsets (q0 - 128*kt). Interior iff 128 <= off <= 1536.
MASK_OFFS = [-384, -256, -128, 0, 1664, 1792, 1920, 2048]

_NC_CACHE = {}


def _phase_a(nc, tc, qk_tts, v_tts, hidT_r, wqkT_r, wvT_r, cosT, sinT):
    """QKV projection (transposed for Q/K, straight for V) + NeoX RoPE."""
    with (
        tc.tile_pool(name="wqk", bufs=1) as wqk_pool,
        tc.tile_pool(name="wv", bufs=1) as wv_pool,
        tc.tile_pool(name="hid", bufs=2) as hid_pool,
        tc.tile_pool(name="cs", bufs=2) as cs_pool,
        tc.tile_pool(name="rope", bufs=4) as rope_pool,
        tc.tile_pool(name="psA", bufs=3, space="PSUM") as psA,
        tc.tile_pool(name="psV", bufs=2, space="PSUM") as psV,
    ):
        KC = KO // 4  # 7-ko chunks so compute starts before all bytes land
        wqk_sbs = []
        for j in range(4):
            w = wqk_pool.tile([128, KC, QK_F], BF16, name=f"wqk{j}")
            nc.sync.dma_start(w, wqkT_r[:, KC * j:KC * (j + 1), :])
            wqk_sbs.append(w)
        wv_sb = wv_pool.tile([128, KO, HD], BF16)
        nc.sync.dma_start(wv_sb, wvT_r)

        for tt in range(TT):
            tsl = bass.ts(tt, 512)
            hid_ts = []
            for j in range(4):
                ht = hid_pool.tile([128, KC, 512], BF16, name=f"hid{j}",
                                   tag=f"hid{j}")
                nc.sync.dma_start(ht, hidT_r[:, KC * j:KC * (j + 1), tsl])
                hid_ts.append(ht)
            cos_t = cs_pool.tile([128, 512], F32, name="cos_t")
            nc.sync.dma_start(cos_t, cosT[:, tsl])
            sin_t = cs_pool.tile([128, 512], F32, name="sin_t")
            nc.sync.dma_start(sin_t, sinT[:, tsl])

            for pair in range(3):
                ps_a = psA.tile([128, 512], F32, name="ps_qk", tag="ps_qk")
                for ko in range(KO):
                    nc.tensor.matmul(
                        ps_a,
                        wqk_sbs[ko // KC][:, ko % KC, bass.ts(2 * pair, 128)],
                        hid_ts[ko // KC][:, ko % KC, :],
                        start=(ko == 0), stop=(ko == KO - 1),
                    )
                ps_b = psA.tile([128, 512], F32, name="ps_qk2", tag="ps_qk")
                for ko in range(KO):
                    nc.tensor.matmul(
                        ps_b,
                        wqk_sbs[ko // KC][:, ko % KC, bass.ts(2 * pair + 1, 128)],
                        hid_ts[ko // KC][:, ko % KC, :],
                        start=(ko == 0), stop=(ko == KO - 1),
                    )
                # NeoX RoPE on the (x1, x2) pair, writing bf16 into qk tile
                t1 = rope_pool.tile([128, 512], F32, name="rp1", tag="rp")
                t2 = rope_pool.tile([128, 512], F32, name="rp2", tag="rp")
                nc.vector.tensor_mul(t1, ps_a, cos_t)
                nc.vector.tensor_mul(t2, ps_b, sin_t)
                nc.vector.tensor_sub(qk_tts[tt][:, 2 * pair, :], t1, t2)
                t3 = rope_pool.tile([128, 512], F32, name="rp3", tag="rp")
                t4 = rope_pool.tile([128, 512], F32, name="rp4", tag="rp")
                nc.vector.tensor_mul(t3, ps_b, cos_t)
                nc.vector.tensor_mul(t4, ps_a, sin_t)
                nc.vector.tensor_add(qk_tts[tt][:, 2 * pair + 1, :], t3, t4)

            for ts4 in range(4):
                ps_v = psV.tile([128, HD], F32, name="ps_v", tag="ps_v")
                for ko in range(KO):
                    nc.tensor.matmul(
                        ps_v,
                        hid_ts[ko // KC][:, ko % KC, bass.ts(ts4, 128)],
                        wv_sb[:, ko, :],
                        start=(ko == 0), stop=(ko == KO - 1),
                    )
                nc.scalar.copy(v_tts[tt][:, ts4, :], ps_v)


def _phase_bc(nc, tc, qk_tts, v_tts, wo_sb, ones_sb, masks_r,
              partials, rs_outs, out):
    """Attention + per-block partial o-proj + chunked ReduceScatter.

    Flat software pipeline over all (qb, h, i) score tiles with LOOK lookahead
    crossing block boundaries. Softmax denominator: bf16 accumulation on the
    vector engine while PE does PV, collapsed per block with one cheap [1,512]
    ones-matmul, reciprocal on [1,512], then gpsimd partition_broadcast. After
    each qb's two heads finish, its 28 o-proj hp-groups are queued and
    drip-fed between attention iterations of the next qb; the per-qb
    ReduceScatter fires as soon as the last partial is in DRAM (split in two
    for the last block to shorten the exposed tail).
    """
    LOOK = 2

    blocks = []
    for qb in range(TT):
        kts = list(range(max(0, 4 * qb - 16), 4 * qb + 4))
        for h in range(2):
            blocks.append((qb, h, kts))
    tasks = []
    for bi, (qb, h, kts) in enumerate(blocks):
        for i in range(len(kts)):
            tasks.append((bi, i))
    # iterations available to drain o-proj of qb (attention iters of qb+1)
    iters_next = {qb: 2 * len(range(max(0, 4 * (qb + 1) - 16), 4 * (qb + 1) + 4))
                  for qb in range(TT - 1)}

    def rs_fire(qb, r0, r1, dst, osl):
        nc.gpsimd.collective_compute(
            "ReduceScatter",
            mybir.AluOpType.add,
            replica_groups=[list(range(N_CORES))],
            ins=[partials[qb][r0:r1, :].opt()],
            outs=[dst.opt()],
        )
        nc.sync.dma_start(out[qb, osl[0]:osl[1], :], dst)

    with (
        tc.tile_pool(name="maskp", bufs=1) as mask_pool,
        tc.tile_pool(name="probs", bufs=6) as probs_pool,
        tc.tile_pool(name="attn", bufs=2) as attn_pool,
        tc.tile_pool(name="dens", bufs=2) as den_pool,
        tc.tile_pool(name="smalls", bufs=2) as small_pool,
        tc.tile_pool(name="psb", bufs=4) as psb_pool,
        tc.tile_pool(name="psS", bufs=3, space="PSUM") as psS,
        tc.tile_pool(name="psO", bufs=1, space="PSUM") as psO,
        tc.tile_pool(name="psD", bufs=1, space="PSUM") as psD,
        tc.tile_pool(name="psC", bufs=2, space="PSUM") as psC,
    ):
        mask_sb = mask_pool.tile([128, 8, 512], BF16)
        nc.sync.dma_start(mask_sb, masks_r)

        state = {}   # bi -> dict with po0/po1/den/probs
        ao_tiles = {}  # qb -> [ao_f0..ao_f3]
        pending = []   # (qb, hp) o-proj groups ready to emit
        pace = [0]

        def scores(t):
            bi, i = tasks[t]
            qb, h, kts = blocks[bi]
            st = state.setdefault(bi, {"probs": {}})
            kt = kts[i]
            q0 = qb * 512
            kq, ks = qk_tts[qb], qk_tts[kt // 4]
            ksl = bass.ts(kt % 4, 128)
            ps = psS.tile([128, 512], F32, name="ps_s", tag="ps_s")
            nc.tensor.matmul(ps, ks[:, 4, ksl], kq[:, 2 * h, :],
                             start=True, stop=False)
            nc.tensor.matmul(ps, ks[:, 5, ksl], kq[:, 2 * h + 1, :],
                             start=False, stop=True)
            pt = probs_pool.tile([128, 512], BF16, name="pt", tag="pt")
            nc.scalar.activation(ps, ps, mybir.ActivationFunctionType.Tanh,
                                 scale=SCALE / SOFTCAP)
            nc.scalar.activation(pt, ps, mybir.ActivationFunctionType.Exp,
                                 scale=SOFTCAP)
            off = q0 - 128 * kt
            if not (128 <= off <= 1536):
                mi = MASK_OFFS.index(off)
                nc.vector.tensor_mul(pt, pt, mask_sb[:, mi, :])
            if i == 0:
                den = den_pool.tile([128, 512], BF16, name="den", tag="den")
                st["den"] = den
                nc.vector.tensor_copy(den, pt)
            else:
                den = st["den"]
                nc.vector.tensor_add(den, den, pt)
            st["probs"][i] = pt

        def av(t):
            bi, i = tasks[t]
            qb, h, kts = blocks[bi]
            st = state[bi]
            n = len(kts)
            kt = kts[i]
            vt = v_tts[kt // 4]
            k4 = kt % 4
            if i == 0:
                st["po0"] = psO.tile([128, 512], F32, name="po0", tag="po0")
                st["po1"] = psO.tile([128, 512], F32, name="po1", tag="po1")
            pt = st["probs"].pop(i)
            first, last = (i == 0), (i == n - 1)
            nc.tensor.matmul(st["po0"], vt[:, k4, 0:128], pt,
                             start=first, stop=last, skip_group_check=True)
            nc.tensor.matmul(st["po1"], vt[:, k4, 128:256], pt,
                             start=first, stop=last, skip_group_check=True)
            if last:
                finalize(bi)

        def finalize(bi):
            qb, h, kts = blocks[bi]
            st = state.pop(bi)
            psd = psD.tile([1, 512], F32, name="psd", tag="psd")
            nc.tensor.matmul(psd, ones_sb, st["den"], start=True, stop=True)
            recip = small_pool.tile([1, 512], F32, name="recip", tag="recip")
            nc.vector.reciprocal(recip, psd)
            rb = small_pool.tile([128, 512], F32, name="rb", tag="rb")
            nc.gpsimd.partition_broadcast(rb, recip)
            aos = ao_tiles.setdefault(qb, [None] * 4)
            for j, po in enumerate((st["po0"], st["po1"])):
                ao = attn_pool.tile([128, 512], BF16, name=f"ao{2 * h + j}",
                                    tag=f"ao{2 * h + j}")
                nc.vector.tensor_mul(ao, po, rb)
                aos[2 * h + j] = ao
            if h == 1:
                pending.extend((qb, hp) for hp in range(HP))
                pace[0] = -(-HP // iters_next.get(qb, 1))
            # Fire qb-2's ReduceScatter here: its partials hit DRAM during
            # qb-1, and spacing the triggers this widely keeps the CC core
            # drained so the gpsimd FIFO (broadcasts) never blocks behind one.
            rsq = qb - 2 if h == 0 else (6 if qb == TT - 1 else None)
            if rsq is not None and rsq >= 0:
                rs_fire(rsq, 0, HID, rs_outs[rsq], (0, HOUT))

        def oproj_group(qb, hp):
            aos = ao_tiles[qb]
            pc = psC.tile([128, 512], F32, name="pc", tag="pc")
            for f in range(4):
                nc.tensor.matmul(pc, wo_sb[:, f, bass.ts(hp, 128)], aos[f],
                                 start=(f == 0), stop=(f == 3),
                                 skip_group_check=True)
            pb = psb_pool.tile([128, 512], BF16, name="pb", tag="pb")
            nc.vector.tensor_copy(pb, pc)
            nc.sync.dma_start(partials[qb][bass.ts(hp, 128), :], pb)
            if qb == TT - 1:
                # tail: split the last block's RS in two hid halves
                if hp == HP // 2 - 1:
                    rs_fire(qb, 0, HID // 2, rs_outs[qb][0], (0, HOUT // 2))
                elif hp == HP - 1:
                    rs_fire(qb, HID // 2, HID, rs_outs[qb][1],
                            (HOUT // 2, HOUT))

        for t in range(min(LOOK, len(tasks))):
            scores(t)
        for t in range(len(tasks)):
            if t + LOOK < len(tasks):
                scores(t + LOOK)
            av(t)
            for _ in range(pace[0]):
                if pending:
                    oproj_group(*pending.pop(0))
        while pending:
            oproj_group(*pending.pop(0))


def build_nc():
    nc = bacc.Bacc()

    hidT = nc.declare_dram_parameter("hidT", [HID, S], BF16, isOutput=False)
    wqkT = nc.declare_dram_parameter("wqkT", [HID, QK_F], BF16, isOutput=False)
    wvT = nc.declare_dram_parameter("wvT", [HID, HD], BF16, isOutput=False)
    woT = nc.declare_dram_parameter("woT", [4 * 128, HID], BF16, isOutput=False)
    cosT = nc.declare_dram_parameter("cosT", [128, S], F32, isOutput=False)
    sinT = nc.declare_dram_parameter("sinT", [128, S], F32, isOutput=False)
    masks = nc.declare_dram_parameter("masks", [8, 128, 512], BF16, isOutput=False)
    out = nc.declare_dram_parameter("out", [TT, HOUT, 512], BF16, isOutput=True)

    hidT_r = hidT.rearrange("(ko p) t -> p ko t", p=128)
    wqkT_r = wqkT.rearrange("(ko p) f -> p ko f", p=128)
    wvT_r = wvT.rearrange("(ko p) d -> p ko d", p=128)
    woT_r = woT.rearrange("(f p) h -> p f h", p=128)
    masks_r = masks.rearrange("m p q -> p m q")

    with tile.TileContext(nc) as tc:
        with (
            tc.tile_pool(name="persist", bufs=1) as persist,
            tc.tile_pool(name="dram", bufs=1, space="DRAM") as dram,
        ):
            partials = [dram.tile([HID, 512], BF16, name=f"partial{qb}")
                        for qb in range(TT)]
            rs_outs = [dram.tile([HOUT, 512], BF16, name=f"rsout{qb}")
                       for qb in range(TT - 1)]
            rs_outs.append([dram.tile([HOUT // 2, 512], BF16, name=f"rsout7{x}")
                            for x in "ab"])

            # live across phases A+B: per-token-tile qk/v so attention reads
            # depend only on the producing tile, not all of phase A
            qk_tts = [persist.tile([128, 6, 512], BF16, name=f"qkt{tt}")
                      for tt in range(TT)]
            v_tts = [persist.tile([128, 4, HD], BF16, name=f"vt{tt}")
                     for tt in range(TT)]
            ones_sb = persist.tile([128, 1], BF16)
            nc.vector.memset(ones_sb, 1.0)

            _phase_a(nc, tc, qk_tts, v_tts, hidT_r, wqkT_r, wvT_r, cosT, sinT)

            with tc.tile_pool(name="wo", bufs=1) as wo_pool:
                wo_sb = wo_pool.tile([128, 4, HID], BF16)
                nc.sync.dma_start(wo_sb, woT_r)
                _phase_bc(nc, tc, qk_tts, v_tts, wo_sb, ones_sb, masks_r,
                          partials, rs_outs, out)

    nc.compile()
    return nc


def get_nc():
    if "nc" not in _NC_CACHE:
        _NC_CACHE["nc"] = build_nc()
    return _NC_CACHE["nc"]


def prep_in_maps(inputs):
    bf16 = ml_dtypes.bfloat16
    hs = np.asarray(inputs["hidden_states"], dtype=np.float32)
    pos = np.asarray(inputs["position_ids"]).reshape(-1).astype(np.float64)
    w_qkv = np.asarray(inputs["w_qkv"], dtype=np.float32)
    w_o = np.asarray(inputs["w_o"], dtype=np.float32)

    hidT = np.ascontiguousarray(hs.reshape(S, HID).T).astype(bf16)

    inv_freq = 1.0 / (THETA ** (np.arange(HD // 2, dtype=np.float64) * 2.0 / HD))
    ang = inv_freq[:, None] * pos[None, :]
    cosT = np.cos(ang).astype(np.float32)
    sinT = np.sin(ang).astype(np.float32)

    kk = np.arange(128)[:, None]
    qq = np.arange(512)[None, :]
    masks = np.stack(
        [((qq - kk + o >= 0) & (qq - kk + o <= WINDOW)) for o in MASK_OFFS]
    ).astype(bf16)

    in_maps = []
    for c in range(N_CORES):
        wq = w_qkv[512 * c:512 * (c + 1)]
        wk = w_qkv[Q_SIZE + HD * c:Q_SIZE + HD * (c + 1)]
        wv = w_qkv[Q_SIZE + NKV * HD + HD * c:Q_SIZE + NKV * HD + HD * (c + 1)]
        wqkT = np.ascontiguousarray(np.concatenate([wq, wk], 0).T).astype(bf16)
        wvT = np.ascontiguousarray(wv.T).astype(bf16)
        woT = np.ascontiguousarray(w_o[:, 512 * c:512 * (c + 1)].T).astype(bf16)
        in_maps.append(
            dict(hidT=hidT, wqkT=wqkT, wvT=wvT, woT=woT,
                 cosT=cosT, sinT=sinT, masks=masks)
        )
    return in_maps


def run(inputs, **kwargs):
    nc = get_nc()
    in_maps = prep_in_maps(inputs)
    return run_bass_kernel_spmd(nc, in_maps, list(range(N_CORES)), **kwargs)


def assemble(res):
    full = np.empty((S, HID), dtype=np.float32)
    HH = HOUT // 2
    for c in range(N_CORES):
        o = np.asarray(res.results[c]["out"], dtype=np.float32)  # [8, 448, 512]
        for qb in range(TT - 1):
            full[qb * 512:(qb + 1) * 512, HOUT * c:HOUT * (c + 1)] = o[qb].T
        # qb 7 was reduce-scattered in two hid halves of 1792 rows each
        rows = slice((TT - 1) * 512, TT * 512)
        full[rows, HH * c:HH * (c + 1)] = o[TT - 1][:HH].T
        full[rows, HID // 2 + HH * c:HID // 2 + HH * (c + 1)] = o[TT - 1][HH:].T
    return full.reshape(1, S, HID)


def kernel(**inputs):
    return assemble(run(inputs))


# revision 20
# speedup vs baseline: 1.0686x; 1.0686x over previous
"""Gemma2 sliding-window attention (B=1, S=4096, HID=3584, 16 Q heads / 8 KV heads,
HD=256, window 2047, tanh softcap 50) on 8 Trainium2 NeuronCores.

Sharding: tensor-parallel over heads. Core c owns Q heads (2c, 2c+1) and KV head c.
  - QKV projection computed transposed ([feature, token] layout) so Q/K land in the
    layout the scores matmul needs with zero on-device transposes. V is computed in
    [token, feature] layout for the PV matmul.
  - Scores are computed transposed ([k, q] tiles); softmax uses no max-subtraction
    (tanh softcap bounds scores to +-50 so exp cannot overflow); the denominator is
    accumulated with a ones-row matmul; masking is multiplicative post-exp with 8
    precomputed boundary masks.
  - Per-core attention outputs ([512 features, 4096 tokens], bf16) are AllGathered,
    then each core computes a 448-column slice of the output projection.
Host side: weights are pre-transposed/pre-cast to bf16, RoPE cos/sin tables are
precomputed from position_ids, outputs are concatenated along the hidden dim.
"""

import sys

if "/opt/trn_rl_repo" not in sys.path:
    sys.path.insert(0, "/opt/trn_rl_repo")

import numpy as np
import ml_dtypes

import concourse.bass as bass
import concourse.tile as tile
from concourse import bacc, mybir
from concourse.bass_utils import run_bass_kernel_spmd

# Problem constants (hardcoded per harness contract)
S = 4096
HID = 3584
NH, NKV, HD = 16, 8, 256
Q_SIZE = NH * HD          # 4096
SCALE = 256.0 ** -0.5     # 1/16
SOFTCAP = 50.0
WINDOW = 2048 - 1         # 2047
THETA = 10000.0

N_CORES = 8
QK_F = 2 * HD + HD        # 768 per-core transposed-qk features: [q_h0, q_h1, k]
HOUT = HID // N_CORES     # 448 output columns per core
KO = HID // 128           # 28 contraction subtiles for projections
TT = S // 512             # 8 token tiles of 512
F32 = mybir.dt.float32
BF16 = mybir.dt.bfloat16

# Boundary-tile diagonal offsets (q0 - 128*kt). Interior iff 128 <= off <= 1536.
MASK_OFFS = [-384, -256, -128, 0, 1664, 1792, 1920, 2048]

_NC_CACHE = {}


def _phase_a(nc, tc, qk_sb, v_sb, hidT_r, wqkT_r, wvT_r, cosT, sinT):
    """QKV projection (transposed for Q/K, straight for V) + NeoX RoPE."""
    with (
        tc.tile_pool(name="wqk", bufs=1) as wqk_pool,
        tc.tile_pool(name="wv", bufs=1) as wv_pool,
        tc.tile_pool(name="hid", bufs=2) as hid_pool,
        tc.tile_pool(name="cs", bufs=2) as cs_pool,
        tc.tile_pool(name="rope", bufs=4) as rope_pool,
        tc.tile_pool(name="psA", bufs=3, space="PSUM") as psA,
        tc.tile_pool(name="psV", bufs=2, space="PSUM") as psV,
    ):
        KC = KO // 4  # 7-ko DMA chunks so compute starts before all bytes land
        wqk_sbs = []
        for j in range(4):
            w = wqk_pool.tile([128, KC, QK_F], BF16, name=f"wqk{j}")
            nc.sync.dma_start(w, wqkT_r[:, KC * j:KC * (j + 1), :])
            wqk_sbs.append(w)
        wv_sb = wv_pool.tile([128, KO, HD], BF16)
        nc.sync.dma_start(wv_sb, wvT_r)

        for tt in range(TT):
            tsl = bass.ts(tt, 512)
            hid_ts = []
            for j in range(4):
                ht = hid_pool.tile([128, KC, 512], BF16, name=f"hid{j}",
                                   tag=f"hid{j}")
                nc.sync.dma_start(ht, hidT_r[:, KC * j:KC * (j + 1), tsl])
                hid_ts.append(ht)
            cos_t = cs_pool.tile([128, 512], F32, name="cos_t")
            nc.sync.dma_start(cos_t, cosT[:, tsl])
            sin_t = cs_pool.tile([128, 512], F32, name="sin_t")
            nc.sync.dma_start(sin_t, sinT[:, tsl])

            for pair in range(3):
                ps_a = psA.tile([128, 512], F32, name="ps_qk", tag="ps_qk")
                for ko in range(KO):
                    nc.tensor.matmul(
                        ps_a,
                        wqk_sbs[ko // KC][:, ko % KC, bass.ts(2 * pair, 128)],
                        hid_ts[ko // KC][:, ko % KC, :],
                        start=(ko == 0), stop=(ko == KO - 1),
                    )
                ps_b = psA.tile([128, 512], F32, name="ps_qk2", tag="ps_qk")
                for ko in range(KO):
                    nc.tensor.matmul(
                        ps_b,
                        wqk_sbs[ko // KC][:, ko % KC, bass.ts(2 * pair + 1, 128)],
                        hid_ts[ko // KC][:, ko % KC, :],
                        start=(ko == 0), stop=(ko == KO - 1),
                    )
                # NeoX RoPE on the (x1, x2) pair, writing bf16 into qk_sb
                t1 = rope_pool.tile([128, 512], F32, name="rp1", tag="rp")
                t2 = rope_pool.tile([128, 512], F32, name="rp2", tag="rp")
                nc.vector.tensor_mul(t1, ps_a, cos_t)
                nc.vector.tensor_mul(t2, ps_b, sin_t)
                nc.vector.tensor_sub(qk_sb[:, 2 * pair, tsl], t1, t2)
                t3 = rope_pool.tile([128, 512], F32, name="rp3", tag="rp")
                t4 = rope_pool.tile([128, 512], F32, name="rp4", tag="rp")
                nc.vector.tensor_mul(t3, ps_b, cos_t)
                nc.vector.tensor_mul(t4, ps_a, sin_t)
                nc.vector.tensor_add(qk_sb[:, 2 * pair + 1, tsl], t3, t4)

            for ts4 in range(4):
                ps_v = psV.tile([128, HD], F32, name="ps_v", tag="ps_v")
                for ko in range(KO):
                    nc.tensor.matmul(
                        ps_v,
                        hid_ts[ko // KC][:, ko % KC, bass.ts(ts4, 128)],
                        wv_sb[:, ko, :],
                        start=(ko == 0), stop=(ko == KO - 1),
                    )
                nc.scalar.copy(v_sb[:, tt * 4 + ts4, :], ps_v)


def _phase_b(nc, tc, qk_sb, v_sb, ones_sb, ag_ins, ag_outs, masks_r):
    """Sliding-window attention with tanh softcap; writes bf16 attnT to ag_ins.

    Token halves: qb 0..3 fill ag_ins[0], qb 4..7 fill ag_ins[1]. The first
    AllGather fires as soon as the first half is done so it overlaps the
    second half's attention compute; the second overlaps phase C's start.
    """
    with (
        tc.tile_pool(name="maskp", bufs=1) as mask_pool,
        tc.tile_pool(name="probs", bufs=8) as probs_pool,
        tc.tile_pool(name="attn", bufs=4) as attn_pool,
        tc.tile_pool(name="smalls", bufs=3) as small_pool,
        tc.tile_pool(name="psS", bufs=3, space="PSUM") as psS,
        tc.tile_pool(name="psO", bufs=2, space="PSUM") as psO,
        tc.tile_pool(name="psD", bufs=1, space="PSUM") as psD,
    ):
        mask_sb = mask_pool.tile([128, 8, 512], BF16)
        nc.sync.dma_start(mask_sb, masks_r)

        # Flat software pipeline across ALL (qb, h) blocks so the scalar
        # engine's tanh/exp latency never drains the PE at block boundaries.
        LOOK = 3
        blocks = []
        for qb in range(TT):
            kts = list(range(max(0, 4 * qb - 16), 4 * qb + 4))
            for h in range(2):
                blocks.append((qb, h, kts))
        tasks = [(bi, i) for bi, (qb, h, kts) in enumerate(blocks)
                 for i in range(len(kts))]
        state = {}

        def scores(t):
            bi, i = tasks[t]
            qb, h, kts = blocks[bi]
            st = state.setdefault(bi, {"probs": {}})
            q0 = qb * 512
            qsl = bass.ts(qb, 512)
            kt = kts[i]
            ksl = bass.ts(kt, 128)
            ps = psS.tile([128, 512], F32, name="ps_s", tag="ps_s")
            nc.tensor.matmul(
                ps, qk_sb[:, 4, ksl], qk_sb[:, 2 * h, qsl],
                start=True, stop=False,
            )
            nc.tensor.matmul(
                ps, qk_sb[:, 5, ksl], qk_sb[:, 2 * h + 1, qsl],
                start=False, stop=True,
            )
            pt = probs_pool.tile([128, 512], BF16, name="pt", tag="pt")
            nc.scalar.activation(
                ps, ps, mybir.ActivationFunctionType.Tanh,
                scale=SCALE / SOFTCAP,
            )
            nc.scalar.activation(
                pt, ps, mybir.ActivationFunctionType.Exp,
                scale=SOFTCAP,
            )
            off = q0 - 128 * kt
            if not (128 <= off <= 1536):
                mi = MASK_OFFS.index(off)
                nc.vector.tensor_mul(pt, pt, mask_sb[:, mi, :])
            st["probs"][i] = pt

        def av(t):
            bi, i = tasks[t]
            qb, h, kts = blocks[bi]
            st = state[bi]
            n = len(kts)
            kt = kts[i]
            if i == 0:
                st["po0"] = psO.tile([128, 512], F32, name="po0", tag="po0")
                st["po1"] = psO.tile([128, 512], F32, name="po1", tag="po1")
                st["pden"] = psD.tile([1, 512], F32, name="pden", tag="pden")
            pt = st["probs"].pop(i)
            first, last = (i == 0), (i == n - 1)
            nc.tensor.matmul(st["po0"], v_sb[:, kt, 0:128], pt,
                             start=first, stop=last, skip_group_check=True)
            nc.tensor.matmul(st["po1"], v_sb[:, kt, 128:256], pt,
                             start=first, stop=last, skip_group_check=True)
            nc.tensor.matmul(st["pden"], ones_sb, pt,
                             start=first, stop=last, skip_group_check=True)
            if last:
                finalize(bi)

        def finalize(bi):
            qb, h, kts = blocks[bi]
            st = state.pop(bi)
            q0 = qb * 512
            recip = small_pool.tile([1, 512], F32, name="recip", tag="recip")
            nc.vector.reciprocal(recip, st["pden"])
            rb = small_pool.tile([128, 512], F32, name="rb", tag="rb")
            nc.gpsimd.partition_broadcast(rb, recip)
            ao0 = attn_pool.tile([128, 512], BF16, name="ao0", tag="ao")
            ao1 = attn_pool.tile([128, 512], BF16, name="ao1", tag="ao")
            nc.vector.tensor_mul(ao0, st["po0"], rb)
            nc.vector.tensor_mul(ao1, st["po1"], rb)
            f0 = h * HD
            ag_in = ag_ins[qb // 4]
            c0 = q0 % 2048
            nc.sync.dma_start(ag_in[f0:f0 + 128, c0:c0 + 512], ao0)
            nc.sync.dma_start(ag_in[f0 + 128:f0 + 256, c0:c0 + 512], ao1)
            if h == 1 and (qb == 3 or qb == TT - 1):
                half = qb // 4
                nc.gpsimd.collective_compute(
                    "AllGather",
                    mybir.AluOpType.bypass,
                    replica_groups=[list(range(N_CORES))],
                    ins=[ag_ins[half].opt()],
                    outs=[ag_outs[half].opt()],
                )

        for t in range(min(LOOK, len(tasks))):
            scores(t)
        for t in range(len(tasks)):
            if t + LOOK < len(tasks):
                scores(t + LOOK)
            av(t)


def _phase_c(nc, tc, wo_sb, ag_outs, out):
    """Output projection: out[:, 448-col slice] = attn_full.T-free matmul."""
    with (
        tc.tile_pool(name="lhs", bufs=6) as lhs_pool,
        tc.tile_pool(name="outp", bufs=4) as out_pool,
        tc.tile_pool(name="psC", bufs=8, space="PSUM") as psC,
    ):
        FO = Q_SIZE // 128  # 32
        for tg in range(TT):
            pcs = [
                psC.tile([128, HOUT], F32, name=f"pc{j}", tag="pc")
                for j in range(4)
            ]
            ag_out = ag_outs[tg // 4]
            for fo in range(FO):
                lt = lhs_pool.tile([128, 512], BF16, name="lt", tag="lt")
                nc.sync.dma_start(lt, ag_out[bass.ts(fo, 128), bass.ts(tg % 4, 512)])
                for j in range(4):
                    nc.tensor.matmul(
                        pcs[j], lt[:, bass.ts(j, 128)], wo_sb[:, fo, :],
                        start=(fo == 0), stop=(fo == FO - 1),
                        skip_group_check=True,
                    )
            for j in range(4):
                ot = out_pool.tile([128, HOUT], F32, name="ot", tag="ot")
                nc.scalar.copy(ot, pcs[j])
                nc.sync.dma_start(out[bass.ts(tg * 4 + j, 128), :], ot)


def build_nc():
    nc = bacc.Bacc()

    hidT = nc.declare_dram_parameter("hidT", [HID, S], BF16, isOutput=False)
    wqkT = nc.declare_dram_parameter("wqkT", [HID, QK_F], BF16, isOutput=False)
    wvT = nc.declare_dram_parameter("wvT", [HID, HD], BF16, isOutput=False)
    woT = nc.declare_dram_parameter("woT", [Q_SIZE, HOUT], BF16, isOutput=False)
    cosT = nc.declare_dram_parameter("cosT", [128, S], F32, isOutput=False)
    sinT = nc.declare_dram_parameter("sinT", [128, S], F32, isOutput=False)
    masks = nc.declare_dram_parameter("masks", [8, 128, 512], BF16, isOutput=False)
    out = nc.declare_dram_parameter("out", [S, HOUT], F32, isOutput=True)

    hidT_r = hidT.rearrange("(ko p) t -> p ko t", p=128)
    wqkT_r = wqkT.rearrange("(ko p) f -> p ko f", p=128)
    wvT_r = wvT.rearrange("(ko p) d -> p ko d", p=128)
    woT_r = woT.rearrange("(fo p) h -> p fo h", p=128)
    masks_r = masks.rearrange("m p q -> p m q")

    with tile.TileContext(nc) as tc:
        with (
            tc.tile_pool(name="persist", bufs=1) as persist,
            tc.tile_pool(name="dram", bufs=1, space="DRAM") as dram,
        ):
            ag_in_a = dram.tile([2 * HD, S // 2], BF16)
            ag_in_b = dram.tile([2 * HD, S // 2], BF16)
            ag_out_a = dram.tile([Q_SIZE, S // 2], BF16, addr_space="Shared")
            ag_out_b = dram.tile([Q_SIZE, S // 2], BF16, addr_space="Shared")
            ag_ins = [ag_in_a, ag_in_b]
            ag_outs = [ag_out_a, ag_out_b]

            # live across phases A+B
            qk_sb = persist.tile([128, 6, S], BF16)   # roped qT/kT rows: [h0x1,h0x2,h1x1,h1x2,kx1,kx2]
            v_sb = persist.tile([128, S // 128, HD], BF16)  # v in [token, d] layout
            ones_sb = persist.tile([128, 1], BF16)
            nc.vector.memset(ones_sb, 1.0)

            _phase_a(nc, tc, qk_sb, v_sb, hidT_r, wqkT_r, wvT_r, cosT, sinT)

            with tc.tile_pool(name="wo", bufs=1) as wo_pool:
                # prefetch o-proj weights during attention
                wo_sb = wo_pool.tile([128, Q_SIZE // 128, HOUT], BF16)
                nc.sync.dma_start(wo_sb, woT_r)

                _phase_b(nc, tc, qk_sb, v_sb, ones_sb, ag_ins, ag_outs, masks_r)
                _phase_c(nc, tc, wo_sb, ag_outs, out)

    nc.compile()
    return nc


def get_nc():
    if "nc" not in _NC_CACHE:
        _NC_CACHE["nc"] = build_nc()
    return _NC_CACHE["nc"]


def prep_in_maps(inputs):
    bf16 = ml_dtypes.bfloat16
    hs = np.asarray(inputs["hidden_states"], dtype=np.float32)
    pos = np.asarray(inputs["position_ids"]).reshape(-1).astype(np.float64)
    w_qkv = np.asarray(inputs["w_qkv"], dtype=np.float32)
    w_o = np.asarray(inputs["w_o"], dtype=np.float32)

    hidT = np.ascontiguousarray(hs.reshape(S, HID).T).astype(bf16)

    inv_freq = 1.0 / (THETA ** (np.arange(HD // 2, dtype=np.float64) * 2.0 / HD))
    ang = inv_freq[:, None] * pos[None, :]
    cosT = np.cos(ang).astype(np.float32)
    sinT = np.sin(ang).astype(np.float32)

    kk = np.arange(128)[:, None]
    qq = np.arange(512)[None, :]
    masks = np.stack(
        [((qq - kk + o >= 0) & (qq - kk + o <= WINDOW)) for o in MASK_OFFS]
    ).astype(bf16)

    in_maps = []
    for c in range(N_CORES):
        wq = w_qkv[512 * c:512 * (c + 1)]
        wk = w_qkv[Q_SIZE + HD * c:Q_SIZE + HD * (c + 1)]
        wv = w_qkv[Q_SIZE + NKV * HD + HD * c:Q_SIZE + NKV * HD + HD * (c + 1)]
        wqkT = np.ascontiguousarray(np.concatenate([wq, wk], 0).T).astype(bf16)
        wvT = np.ascontiguousarray(wv.T).astype(bf16)
        woT = np.ascontiguousarray(w_o[HOUT * c:HOUT * (c + 1)].T).astype(bf16)
        in_maps.append(
            dict(hidT=hidT, wqkT=wqkT, wvT=wvT, woT=woT,
                 cosT=cosT, sinT=sinT, masks=masks)
        )
    return in_maps


def run(inputs, **kwargs):
    nc = get_nc()
    in_maps = prep_in_maps(inputs)
    return run_bass_kernel_spmd(nc, in_maps, list(range(N_CORES)), **kwargs)


def assemble(res):
    outs = [res.results[c]["out"] for c in range(N_CORES)]
    full = np.concatenate(outs, axis=1).astype(np.float32)
    return full.reshape(1, S, HID)


def kernel(**inputs):
    return assemble(run(inputs))



# revision 24
# speedup vs baseline: 1.0815x; 1.0121x over previous
"""Gemma2 sliding-window attention (B=1, S=4096, HID=3584, 16 Q heads / 8 KV heads,
HD=256, window 2047, tanh softcap 50) on 8 Trainium2 NeuronCores.

Sharding: tensor-parallel over heads. Core c owns Q heads (2c, 2c+1) and KV head c.
  - QKV projection computed transposed ([feature, token] layout) so Q/K land in the
    layout the scores matmul needs with zero on-device transposes. V is computed in
    [token, feature] layout for the PV matmul.
  - Scores are computed transposed ([k, q] tiles); softmax uses no max-subtraction
    (tanh softcap bounds scores to +-50 so exp cannot overflow); the denominator is
    accumulated with a ones-row matmul; masking is multiplicative post-exp with 8
    precomputed boundary masks.
  - Per-core attention outputs ([512 features, 4096 tokens], bf16) are AllGathered,
    then each core computes a 448-column slice of the output projection.
Host side: weights are pre-transposed/pre-cast to bf16, RoPE cos/sin tables are
precomputed from position_ids, outputs are concatenated along the hidden dim.
"""

import sys

if "/opt/trn_rl_repo" not in sys.path:
    sys.path.insert(0, "/opt/trn_rl_repo")

import numpy as np
import ml_dtypes

import concourse.bass as bass
import concourse.tile as tile
from concourse import bacc, mybir
from concourse.bass_utils import run_bass_kernel_spmd

# Problem constants (hardcoded per harness contract)
S = 4096
HID = 3584
NH, NKV, HD = 16, 8, 256
Q_SIZE = NH * HD          # 4096
SCALE = 256.0 ** -0.5     # 1/16
SOFTCAP = 50.0
WINDOW = 2048 - 1         # 2047
THETA = 10000.0

N_CORES = 8
QK_F = 2 * HD + HD        # 768 per-core transposed-qk features: [q_h0, q_h1, k]
HOUT = HID // N_CORES     # 448 output columns per core
KO = HID // 128           # 28 contraction subtiles for projections
TT = S // 512             # 8 token tiles of 512
F32 = mybir.dt.float32
BF16 = mybir.dt.bfloat16

# Boundary-tile diagonal offsets (q0 - 128*kt). Interior iff 128 <= off <= 1536.
MASK_OFFS = [-384, -256, -128, 0, 1664, 1792, 1920, 2048]

_NC_CACHE = {}


def _phase_a(nc, tc, qk_sb, v_sb, hidT_r, wqkT_r, wvT_r, cosT, sinT):
    """QKV projection (transposed for Q/K, straight for V) + NeoX RoPE."""
    with (
        tc.tile_pool(name="wqk", bufs=1) as wqk_pool,
        tc.tile_pool(name="wv", bufs=1) as wv_pool,
        tc.tile_pool(name="hid", bufs=2) as hid_pool,
        tc.tile_pool(name="cs", bufs=2) as cs_pool,
        tc.tile_pool(name="rope", bufs=4) as rope_pool,
        tc.tile_pool(name="psA", bufs=3, space="PSUM") as psA,
        tc.tile_pool(name="psV", bufs=2, space="PSUM") as psV,
    ):
        KC = KO // 4  # 7-ko DMA chunks so compute starts before all bytes land
        wqk_sbs = []
        for j in range(4):
            w = wqk_pool.tile([128, KC, QK_F], BF16, name=f"wqk{j}")
            nc.sync.dma_start(w, wqkT_r[:, KC * j:KC * (j + 1), :])
            wqk_sbs.append(w)
        wv_sb = wv_pool.tile([128, KO, HD], BF16)
        nc.sync.dma_start(wv_sb, wvT_r)

        for tt in range(TT):
            tsl = bass.ts(tt, 512)
            hid_ts = []
            for j in range(4):
                ht = hid_pool.tile([128, KC, 512], BF16, name=f"hid{j}",
                                   tag=f"hid{j}")
                nc.sync.dma_start(ht, hidT_r[:, KC * j:KC * (j + 1), tsl])
                hid_ts.append(ht)
            cos_t = cs_pool.tile([128, 512], F32, name="cos_t")
            nc.sync.dma_start(cos_t, cosT[:, tsl])
            sin_t = cs_pool.tile([128, 512], F32, name="sin_t")
            nc.sync.dma_start(sin_t, sinT[:, tsl])

            for pair in range(3):
                ps_a = psA.tile([128, 512], F32, name="ps_qk", tag="ps_qk")
                for ko in range(KO):
                    nc.tensor.matmul(
                        ps_a,
                        wqk_sbs[ko // KC][:, ko % KC, bass.ts(2 * pair, 128)],
                        hid_ts[ko // KC][:, ko % KC, :],
                        start=(ko == 0), stop=(ko == KO - 1),
                    )
                ps_b = psA.tile([128, 512], F32, name="ps_qk2", tag="ps_qk")
                for ko in range(KO):
                    nc.tensor.matmul(
                        ps_b,
                        wqk_sbs[ko // KC][:, ko % KC, bass.ts(2 * pair + 1, 128)],
                        hid_ts[ko // KC][:, ko % KC, :],
                        start=(ko == 0), stop=(ko == KO - 1),
                    )
                # NeoX RoPE on the (x1, x2) pair, writing bf16 into qk_sb
                t1 = rope_pool.tile([128, 512], F32, name="rp1", tag="rp")
                t2 = rope_pool.tile([128, 512], F32, name="rp2", tag="rp")
                nc.vector.tensor_mul(t1, ps_a, cos_t)
                nc.vector.tensor_mul(t2, ps_b, sin_t)
                nc.vector.tensor_sub(qk_sb[:, 2 * pair, tsl], t1, t2)
                t3 = rope_pool.tile([128, 512], F32, name="rp3", tag="rp")
                t4 = rope_pool.tile([128, 512], F32, name="rp4", tag="rp")
                nc.vector.tensor_mul(t3, ps_b, cos_t)
                nc.vector.tensor_mul(t4, ps_a, sin_t)
                nc.vector.tensor_add(qk_sb[:, 2 * pair + 1, tsl], t3, t4)

            for ts4 in range(4):
                ps_v = psV.tile([128, HD], F32, name="ps_v", tag="ps_v")
                for ko in range(KO):
                    nc.tensor.matmul(
                        ps_v,
                        hid_ts[ko // KC][:, ko % KC, bass.ts(ts4, 128)],
                        wv_sb[:, ko, :],
                        start=(ko == 0), stop=(ko == KO - 1),
                    )
                nc.scalar.copy(v_sb[:, tt * 4 + ts4, :], ps_v)


def _phase_b(nc, tc, qk_sb, v_sb, ones_sb, ag_ins, ag_outs, masks_r):
    """Sliding-window attention with tanh softcap; writes bf16 attnT to ag_ins.

    Token halves: qb 0..3 fill ag_ins[0], qb 4..7 fill ag_ins[1]. The first
    AllGather fires as soon as the first half is done so it overlaps the
    second half's attention compute; the second overlaps phase C's start.
    """
    with (
        tc.tile_pool(name="maskp", bufs=1) as mask_pool,
        tc.tile_pool(name="probs", bufs=8) as probs_pool,
        tc.tile_pool(name="attn", bufs=4) as attn_pool,
        tc.tile_pool(name="smalls", bufs=3) as small_pool,
        tc.tile_pool(name="psS", bufs=3, space="PSUM") as psS,
        tc.tile_pool(name="psO", bufs=2, space="PSUM") as psO,
        tc.tile_pool(name="psD", bufs=1, space="PSUM") as psD,
    ):
        mask_sb = mask_pool.tile([128, 8, 512], BF16)
        nc.sync.dma_start(mask_sb, masks_r)

        # Flat software pipeline across ALL (qb, h) blocks so the scalar
        # engine's tanh/exp latency never drains the PE at block boundaries.
        LOOK = 3
        blocks = []
        # Descending qb order: the big trailing blocks run first, so each
        # 2-qb AllGather chunk fires long before phase C consumes it and no
        # collective latency is ever exposed.
        for qb in range(TT - 1, -1, -1):
            kts = list(range(max(0, 4 * qb - 16), 4 * qb + 4))
            for h in range(2):
                blocks.append((qb, h, kts))
        tasks = [(bi, i) for bi, (qb, h, kts) in enumerate(blocks)
                 for i in range(len(kts))]
        state = {}

        def scores(t):
            bi, i = tasks[t]
            qb, h, kts = blocks[bi]
            st = state.setdefault(bi, {"probs": {}})
            q0 = qb * 512
            qsl = bass.ts(qb, 512)
            kt = kts[i]
            ksl = bass.ts(kt, 128)
            ps = psS.tile([128, 512], F32, name="ps_s", tag="ps_s")
            nc.tensor.matmul(
                ps, qk_sb[:, 4, ksl], qk_sb[:, 2 * h, qsl],
                start=True, stop=False,
            )
            nc.tensor.matmul(
                ps, qk_sb[:, 5, ksl], qk_sb[:, 2 * h + 1, qsl],
                start=False, stop=True,
            )
            pt = probs_pool.tile([128, 512], BF16, name="pt", tag="pt")
            nc.scalar.activation(
                ps, ps, mybir.ActivationFunctionType.Tanh,
                scale=SCALE / SOFTCAP,
            )
            nc.scalar.activation(
                pt, ps, mybir.ActivationFunctionType.Exp,
                scale=SOFTCAP,
            )
            off = q0 - 128 * kt
            if not (128 <= off <= 1536):
                mi = MASK_OFFS.index(off)
                nc.vector.tensor_mul(pt, pt, mask_sb[:, mi, :])
            st["probs"][i] = pt

        def av(t):
            bi, i = tasks[t]
            qb, h, kts = blocks[bi]
            st = state[bi]
            n = len(kts)
            kt = kts[i]
            if i == 0:
                st["po0"] = psO.tile([128, 512], F32, name="po0", tag="po0")
                st["po1"] = psO.tile([128, 512], F32, name="po1", tag="po1")
                st["pden"] = psD.tile([1, 512], F32, name="pden", tag="pden")
            pt = st["probs"].pop(i)
            first, last = (i == 0), (i == n - 1)
            nc.tensor.matmul(st["po0"], v_sb[:, kt, 0:128], pt,
                             start=first, stop=last, skip_group_check=True)
            nc.tensor.matmul(st["po1"], v_sb[:, kt, 128:256], pt,
                             start=first, stop=last, skip_group_check=True)
            nc.tensor.matmul(st["pden"], ones_sb, pt,
                             start=first, stop=last, skip_group_check=True)
            if last:
                finalize(bi)

        def finalize(bi):
            qb, h, kts = blocks[bi]
            st = state.pop(bi)
            q0 = qb * 512
            recip = small_pool.tile([1, 512], F32, name="recip", tag="recip")
            nc.vector.reciprocal(recip, st["pden"])
            rb = small_pool.tile([128, 512], F32, name="rb", tag="rb")
            nc.gpsimd.partition_broadcast(rb, recip)
            ao0 = attn_pool.tile([128, 512], BF16, name="ao0", tag="ao")
            ao1 = attn_pool.tile([128, 512], BF16, name="ao1", tag="ao")
            nc.vector.tensor_mul(ao0, st["po0"], rb)
            nc.vector.tensor_mul(ao1, st["po1"], rb)
            f0 = h * HD
            ag_in = ag_ins[qb // 2]
            c0 = q0 % 1024
            nc.sync.dma_start(ag_in[f0:f0 + 128, c0:c0 + 512], ao0)
            nc.sync.dma_start(ag_in[f0 + 128:f0 + 256, c0:c0 + 512], ao1)
            # qb pair (2k+1, 2k) runs in that order; at (2k, h1) both halves
            # of chunk k are in DRAM -> fire its AllGather
            if h == 1 and qb % 2 == 0:
                chunk = qb // 2
                nc.gpsimd.collective_compute(
                    "AllGather",
                    mybir.AluOpType.bypass,
                    replica_groups=[list(range(N_CORES))],
                    ins=[ag_ins[chunk].opt()],
                    outs=[ag_outs[chunk].opt()],
                )

        for t in range(min(LOOK, len(tasks))):
            scores(t)
        for t in range(len(tasks)):
            if t + LOOK < len(tasks):
                scores(t + LOOK)
            av(t)


def _phase_c(nc, tc, wo_sb, ag_outs, out):
    """Output projection: out[:, 448-col slice] = attn_full.T-free matmul."""
    with (
        tc.tile_pool(name="lhs", bufs=6) as lhs_pool,
        tc.tile_pool(name="outp", bufs=4) as out_pool,
        tc.tile_pool(name="psC", bufs=8, space="PSUM") as psC,
    ):
        FO = Q_SIZE // 128  # 32
        # chunk-completion order: AG(3) fired first, AG(0) last
        for tg in (6, 7, 4, 5, 2, 3, 0, 1):
            pcs = [
                psC.tile([128, HOUT], F32, name=f"pc{j}", tag="pc")
                for j in range(4)
            ]
            ag_out = ag_outs[tg // 2]
            for fo in range(FO):
                lt = lhs_pool.tile([128, 512], BF16, name="lt", tag="lt")
                nc.sync.dma_start(lt, ag_out[bass.ts(fo, 128), bass.ts(tg % 2, 512)])
                for j in range(4):
                    nc.tensor.matmul(
                        pcs[j], lt[:, bass.ts(j, 128)], wo_sb[:, fo, :],
                        start=(fo == 0), stop=(fo == FO - 1),
                        skip_group_check=True,
                    )
            for j in range(4):
                ot = out_pool.tile([128, HOUT], F32, name="ot", tag="ot")
                nc.scalar.copy(ot, pcs[j])
                nc.sync.dma_start(out[bass.ts(tg * 4 + j, 128), :], ot)


def build_nc():
    nc = bacc.Bacc()

    hidT = nc.declare_dram_parameter("hidT", [HID, S], BF16, isOutput=False)
    wqkT = nc.declare_dram_parameter("wqkT", [HID, QK_F], BF16, isOutput=False)
    wvT = nc.declare_dram_parameter("wvT", [HID, HD], BF16, isOutput=False)
    woT = nc.declare_dram_parameter("woT", [Q_SIZE, HOUT], BF16, isOutput=False)
    cosT = nc.declare_dram_parameter("cosT", [128, S], F32, isOutput=False)
    sinT = nc.declare_dram_parameter("sinT", [128, S], F32, isOutput=False)
    masks = nc.declare_dram_parameter("masks", [8, 128, 512], BF16, isOutput=False)
    out = nc.declare_dram_parameter("out", [S, HOUT], F32, isOutput=True)

    hidT_r = hidT.rearrange("(ko p) t -> p ko t", p=128)
    wqkT_r = wqkT.rearrange("(ko p) f -> p ko f", p=128)
    wvT_r = wvT.rearrange("(ko p) d -> p ko d", p=128)
    woT_r = woT.rearrange("(fo p) h -> p fo h", p=128)
    masks_r = masks.rearrange("m p q -> p m q")

    with tile.TileContext(nc) as tc:
        with (
            tc.tile_pool(name="persist", bufs=1) as persist,
            tc.tile_pool(name="dram", bufs=1, space="DRAM") as dram,
        ):
            ag_ins = [dram.tile([2 * HD, S // 4], BF16, name=f"agi{k}")
                      for k in range(4)]
            ag_outs = [dram.tile([Q_SIZE, S // 4], BF16, name=f"ago{k}",
                                 addr_space="Shared") for k in range(4)]

            # live across phases A+B
            qk_sb = persist.tile([128, 6, S], BF16)   # roped qT/kT rows: [h0x1,h0x2,h1x1,h1x2,kx1,kx2]
            v_sb = persist.tile([128, S // 128, HD], BF16)  # v in [token, d] layout
            ones_sb = persist.tile([128, 1], BF16)
            nc.vector.memset(ones_sb, 1.0)

            _phase_a(nc, tc, qk_sb, v_sb, hidT_r, wqkT_r, wvT_r, cosT, sinT)

            with tc.tile_pool(name="wo", bufs=1) as wo_pool:
                # prefetch o-proj weights during attention
                wo_sb = wo_pool.tile([128, Q_SIZE // 128, HOUT], BF16)
                nc.sync.dma_start(wo_sb, woT_r)

                _phase_b(nc, tc, qk_sb, v_sb, ones_sb, ag_ins, ag_outs, masks_r)
                _phase_c(nc, tc, wo_sb, ag_outs, out)

    nc.compile()
    return nc


def get_nc():
    if "nc" not in _NC_CACHE:
        _NC_CACHE["nc"] = build_nc()
    return _NC_CACHE["nc"]


def prep_in_maps(inputs):
    bf16 = ml_dtypes.bfloat16
    hs = np.asarray(inputs["hidden_states"], dtype=np.float32)
    pos = np.asarray(inputs["position_ids"]).reshape(-1).astype(np.float64)
    w_qkv = np.asarray(inputs["w_qkv"], dtype=np.float32)
    w_o = np.asarray(inputs["w_o"], dtype=np.float32)

    hidT = np.ascontiguousarray(hs.reshape(S, HID).T).astype(bf16)

    inv_freq = 1.0 / (THETA ** (np.arange(HD // 2, dtype=np.float64) * 2.0 / HD))
    ang = inv_freq[:, None] * pos[None, :]
    cosT = np.cos(ang).astype(np.float32)
    sinT = np.sin(ang).astype(np.float32)

    kk = np.arange(128)[:, None]
    qq = np.arange(512)[None, :]
    masks = np.stack(
        [((qq - kk + o >= 0) & (qq - kk + o <= WINDOW)) for o in MASK_OFFS]
    ).astype(bf16)

    in_maps = []
    for c in range(N_CORES):
        wq = w_qkv[512 * c:512 * (c + 1)]
        wk = w_qkv[Q_SIZE + HD * c:Q_SIZE + HD * (c + 1)]
        wv = w_qkv[Q_SIZE + NKV * HD + HD * c:Q_SIZE + NKV * HD + HD * (c + 1)]
        wqkT = np.ascontiguousarray(np.concatenate([wq, wk], 0).T).astype(bf16)
        wvT = np.ascontiguousarray(wv.T).astype(bf16)
        woT = np.ascontiguousarray(w_o[HOUT * c:HOUT * (c + 1)].T).astype(bf16)
        in_maps.append(
            dict(hidT=hidT, wqkT=wqkT, wvT=wvT, woT=woT,
                 cosT=cosT, sinT=sinT, masks=masks)
        )
    return in_maps


def run(inputs, **kwargs):
    nc = get_nc()
    in_maps = prep_in_maps(inputs)
    return run_bass_kernel_spmd(nc, in_maps, list(range(N_CORES)), **kwargs)


def assemble(res):
    outs = [res.results[c]["out"] for c in range(N_CORES)]
    full = np.concatenate(outs, axis=1).astype(np.float32)
    return full.reshape(1, S, HID)


def kernel(**inputs):
    return assemble(run(inputs))



# revision 26
# speedup vs baseline: 1.1422x; 1.0561x over previous
"""Gemma2 sliding-window attention (B=1, S=4096, HID=3584, 16 Q heads / 8 KV heads,
HD=256, window 2047, tanh softcap 50) on 8 Trainium2 NeuronCores.

Sharding: tensor-parallel over heads. Core c owns Q heads (2c, 2c+1) and KV head c.
  - QKV projection computed transposed ([feature, token] layout) so Q/K land in the
    layout the scores matmul needs with zero on-device transposes. V is computed in
    [token, feature] layout for the PV matmul.
  - Scores are computed transposed ([k, q] tiles); softmax uses no max-subtraction
    (tanh softcap bounds scores to +-50 so exp cannot overflow); the denominator is
    accumulated with a ones-row matmul; masking is multiplicative post-exp with 8
    precomputed boundary masks.
  - Per-core attention outputs ([512 features, 4096 tokens], bf16) are AllGathered,
    then each core computes a 448-column slice of the output projection.
Host side: weights are pre-transposed/pre-cast to bf16, RoPE cos/sin tables are
precomputed from position_ids, outputs are concatenated along the hidden dim.
"""

import sys

if "/opt/trn_rl_repo" not in sys.path:
    sys.path.insert(0, "/opt/trn_rl_repo")

import numpy as np
import ml_dtypes

import concourse.bass as bass
import concourse.tile as tile
from concourse import bacc, mybir
from concourse.bass_utils import run_bass_kernel_spmd

# Problem constants (hardcoded per harness contract)
S = 4096
HID = 3584
NH, NKV, HD = 16, 8, 256
Q_SIZE = NH * HD          # 4096
SCALE = 256.0 ** -0.5     # 1/16
SOFTCAP = 50.0
WINDOW = 2048 - 1         # 2047
THETA = 10000.0

N_CORES = 8
QK_F = 2 * HD + HD        # 768 per-core transposed-qk features: [q_h0, q_h1, k]
HOUT = HID // N_CORES     # 448 output columns per core
KO = HID // 128           # 28 contraction subtiles for projections
TT = S // 512             # 8 token tiles of 512
F32 = mybir.dt.float32
BF16 = mybir.dt.bfloat16

# Boundary-tile diagonal offsets (q0 - 128*kt). Interior iff 128 <= off <= 1536.
MASK_OFFS = [-384, -256, -128, 0, 1664, 1792, 1920, 2048]

_NC_CACHE = {}


def _phase_a(nc, tc, qk_sb, v_sb, hidT_r, wqkT_r, wvT_r, cosT, sinT):
    """QKV projection (transposed for Q/K, straight for V) + NeoX RoPE."""
    with (
        tc.tile_pool(name="wqk", bufs=1) as wqk_pool,
        tc.tile_pool(name="wv", bufs=1) as wv_pool,
        tc.tile_pool(name="hid", bufs=2) as hid_pool,
        tc.tile_pool(name="cs", bufs=2) as cs_pool,
        tc.tile_pool(name="rope", bufs=4) as rope_pool,
        tc.tile_pool(name="psA", bufs=3, space="PSUM") as psA,
        tc.tile_pool(name="psV", bufs=2, space="PSUM") as psV,
    ):
        KC = KO // 4  # 7-ko DMA chunks so compute starts before all bytes land
        wqk_sbs = []
        for j in range(4):
            w = wqk_pool.tile([128, KC, QK_F], BF16, name=f"wqk{j}")
            nc.sync.dma_start(w, wqkT_r[:, KC * j:KC * (j + 1), :])
            wqk_sbs.append(w)
        wv_sb = wv_pool.tile([128, KO, HD], BF16)
        nc.sync.dma_start(wv_sb, wvT_r)

        for tt in range(TT):
            tsl = bass.ts(tt, 512)
            hid_ts = []
            for j in range(4):
                ht = hid_pool.tile([128, KC, 512], BF16, name=f"hid{j}",
                                   tag=f"hid{j}")
                nc.sync.dma_start(ht, hidT_r[:, KC * j:KC * (j + 1), tsl])
                hid_ts.append(ht)
            cos_t = cs_pool.tile([128, 512], F32, name="cos_t")
            nc.sync.dma_start(cos_t, cosT[:, tsl])
            sin_t = cs_pool.tile([128, 512], F32, name="sin_t")
            nc.sync.dma_start(sin_t, sinT[:, tsl])

            for pair in range(3):
                ps_a = psA.tile([128, 512], F32, name="ps_qk", tag="ps_qk")
                for ko in range(KO):
                    nc.tensor.matmul(
                        ps_a,
                        wqk_sbs[ko // KC][:, ko % KC, bass.ts(2 * pair, 128)],
                        hid_ts[ko // KC][:, ko % KC, :],
                        start=(ko == 0), stop=(ko == KO - 1),
                    )
                ps_b = psA.tile([128, 512], F32, name="ps_qk2", tag="ps_qk")
                for ko in range(KO):
                    nc.tensor.matmul(
                        ps_b,
                        wqk_sbs[ko // KC][:, ko % KC, bass.ts(2 * pair + 1, 128)],
                        hid_ts[ko // KC][:, ko % KC, :],
                        start=(ko == 0), stop=(ko == KO - 1),
                    )
                # NeoX RoPE on the (x1, x2) pair, writing bf16 into qk_sb
                t1 = rope_pool.tile([128, 512], F32, name="rp1", tag="rp")
                t2 = rope_pool.tile([128, 512], F32, name="rp2", tag="rp")
                nc.vector.tensor_mul(t1, ps_a, cos_t)
                nc.vector.tensor_mul(t2, ps_b, sin_t)
                nc.vector.tensor_sub(qk_sb[:, 2 * pair, tsl], t1, t2)
                t3 = rope_pool.tile([128, 512], F32, name="rp3", tag="rp")
                t4 = rope_pool.tile([128, 512], F32, name="rp4", tag="rp")
                nc.vector.tensor_mul(t3, ps_b, cos_t)
                nc.vector.tensor_mul(t4, ps_a, sin_t)
                nc.vector.tensor_add(qk_sb[:, 2 * pair + 1, tsl], t3, t4)

            for ts4 in range(4):
                ps_v = psV.tile([128, HD], F32, name="ps_v", tag="ps_v")
                for ko in range(KO):
                    nc.tensor.matmul(
                        ps_v,
                        hid_ts[ko // KC][:, ko % KC, bass.ts(ts4, 128)],
                        wv_sb[:, ko, :],
                        start=(ko == 0), stop=(ko == KO - 1),
                    )
                nc.scalar.copy(v_sb[:, tt * 4 + ts4, :], ps_v)


def _phase_b(nc, tc, qk_sb, v_sb, ones_sb, ag_ins, ag_outs, masks_r):
    """Sliding-window attention with tanh softcap; writes bf16 attnT to ag_ins.

    Token halves: qb 0..3 fill ag_ins[0], qb 4..7 fill ag_ins[1]. The first
    AllGather fires as soon as the first half is done so it overlaps the
    second half's attention compute; the second overlaps phase C's start.
    """
    with (
        tc.tile_pool(name="maskp", bufs=1) as mask_pool,
        tc.tile_pool(name="probs", bufs=8) as probs_pool,
        tc.tile_pool(name="attn", bufs=4) as attn_pool,
        tc.tile_pool(name="smalls", bufs=3) as small_pool,
        tc.tile_pool(name="psS", bufs=3, space="PSUM") as psS,
        tc.tile_pool(name="psO", bufs=2, space="PSUM") as psO,
        tc.tile_pool(name="psD", bufs=1, space="PSUM") as psD,
    ):
        mask_sb = mask_pool.tile([128, 8, 512], BF16)
        nc.sync.dma_start(mask_sb, masks_r)

        # Flat software pipeline across ALL (qb, h) blocks so the scalar
        # engine's tanh/exp latency never drains the PE at block boundaries.
        LOOK = 3
        blocks = []
        # Descending qb order: the big trailing blocks run first, so each
        # 2-qb AllGather chunk fires long before phase C consumes it and no
        # collective latency is ever exposed.
        for qb in range(TT - 1, -1, -1):
            kts = list(range(max(0, 4 * qb - 16), 4 * qb + 4))
            for h in range(2):
                blocks.append((qb, h, kts))
        tasks = [(bi, i) for bi, (qb, h, kts) in enumerate(blocks)
                 for i in range(len(kts))]
        state = {}

        def scores(t):
            bi, i = tasks[t]
            qb, h, kts = blocks[bi]
            st = state.setdefault(bi, {"probs": {}})
            q0 = qb * 512
            qsl = bass.ts(qb, 512)
            kt = kts[i]
            ksl = bass.ts(kt, 128)
            ps = psS.tile([128, 512], F32, name="ps_s", tag="ps_s")
            nc.tensor.matmul(
                ps, qk_sb[:, 4, ksl], qk_sb[:, 2 * h, qsl],
                start=True, stop=False,
            )
            nc.tensor.matmul(
                ps, qk_sb[:, 5, ksl], qk_sb[:, 2 * h + 1, qsl],
                start=False, stop=True,
            )
            pt = probs_pool.tile([128, 512], BF16, name="pt", tag="pt")
            nc.scalar.activation(
                ps, ps, mybir.ActivationFunctionType.Tanh,
                scale=SCALE / SOFTCAP,
            )
            nc.scalar.activation(
                pt, ps, mybir.ActivationFunctionType.Exp,
                scale=SOFTCAP,
            )
            off = q0 - 128 * kt
            if not (128 <= off <= 1536):
                mi = MASK_OFFS.index(off)
                nc.vector.tensor_mul(pt, pt, mask_sb[:, mi, :])
            st["probs"][i] = pt

        def av(t):
            bi, i = tasks[t]
            qb, h, kts = blocks[bi]
            st = state[bi]
            n = len(kts)
            kt = kts[i]
            if i == 0:
                st["po0"] = psO.tile([128, 512], F32, name="po0", tag="po0")
                st["po1"] = psO.tile([128, 512], F32, name="po1", tag="po1")
                st["pden"] = psD.tile([1, 512], F32, name="pden", tag="pden")
            pt = st["probs"].pop(i)
            first, last = (i == 0), (i == n - 1)
            nc.tensor.matmul(st["po0"], v_sb[:, kt, 0:128], pt,
                             start=first, stop=last, skip_group_check=True)
            nc.tensor.matmul(st["po1"], v_sb[:, kt, 128:256], pt,
                             start=first, stop=last, skip_group_check=True)
            nc.tensor.matmul(st["pden"], ones_sb, pt,
                             start=first, stop=last, skip_group_check=True)
            if last:
                finalize(bi)

        def finalize(bi):
            qb, h, kts = blocks[bi]
            st = state.pop(bi)
            q0 = qb * 512
            recip = small_pool.tile([1, 512], F32, name="recip", tag="recip")
            nc.vector.reciprocal(recip, st["pden"])
            rb = small_pool.tile([128, 512], F32, name="rb", tag="rb")
            nc.gpsimd.partition_broadcast(rb, recip)
            ao0 = attn_pool.tile([128, 512], BF16, name="ao0", tag="ao")
            ao1 = attn_pool.tile([128, 512], BF16, name="ao1", tag="ao")
            nc.vector.tensor_mul(ao0, st["po0"], rb)
            nc.vector.tensor_mul(ao1, st["po1"], rb)
            f0 = h * HD
            ag_in = ag_ins[qb // 2]
            c0 = q0 % 1024
            nc.sync.dma_start(ag_in[f0:f0 + 128, c0:c0 + 512], ao0)
            nc.sync.dma_start(ag_in[f0 + 128:f0 + 256, c0:c0 + 512], ao1)
            # qb pair (2k+1, 2k) runs in that order; at (2k, h1) both halves
            # of chunk k are in DRAM -> fire its AllGather
            if h == 1 and qb % 2 == 0:
                chunk = qb // 2
                nc.gpsimd.collective_compute(
                    "AllGather",
                    mybir.AluOpType.bypass,
                    replica_groups=[list(range(N_CORES))],
                    ins=[ag_ins[chunk].opt()],
                    outs=[ag_outs[chunk].opt()],
                )

        for t in range(min(LOOK, len(tasks))):
            scores(t)
        for t in range(len(tasks)):
            bi, i = tasks[t]
            block_last = i == len(blocks[bi][2]) - 1
            # Emit a block-closing av (whose finalize queues the reciprocal /
            # broadcast / normalize chain) BEFORE the lookahead scores, so the
            # next block's pden matmul isn't stalled behind lookahead DVE work.
            if block_last:
                av(t)
            if t + LOOK < len(tasks):
                scores(t + LOOK)
            if not block_last:
                av(t)


def _phase_c(nc, tc, wo_sb, ag_outs, out):
    """Output projection: out[:, 448-col slice] = attn_full.T-free matmul."""
    with (
        tc.tile_pool(name="lhs", bufs=10) as lhs_pool,
        tc.tile_pool(name="outp", bufs=4) as out_pool,
        tc.tile_pool(name="psC", bufs=8, space="PSUM") as psC,
    ):
        FO = Q_SIZE // 128  # 32
        # chunk-completion order: AG(3) fired first, AG(0) last
        for tg in (6, 7, 4, 5, 2, 3, 0, 1):
            pcs = [
                psC.tile([128, HOUT], F32, name=f"pc{j}", tag="pc")
                for j in range(4)
            ]
            ag_out = ag_outs[tg // 2]
            for fo in range(FO):
                lt = lhs_pool.tile([128, 512], BF16, name="lt", tag="lt")
                nc.sync.dma_start(lt, ag_out[bass.ts(fo, 128), bass.ts(tg % 2, 512)])
                for j in range(4):
                    nc.tensor.matmul(
                        pcs[j], lt[:, bass.ts(j, 128)], wo_sb[:, fo, :],
                        start=(fo == 0), stop=(fo == FO - 1),
                        skip_group_check=True,
                    )
            for j in range(4):
                ot = out_pool.tile([128, HOUT], F32, name="ot", tag="ot")
                nc.scalar.copy(ot, pcs[j])
                nc.sync.dma_start(out[bass.ts(tg * 4 + j, 128), :], ot)


def build_nc():
    nc = bacc.Bacc()

    hidT = nc.declare_dram_parameter("hidT", [HID, S], BF16, isOutput=False)
    wqkT = nc.declare_dram_parameter("wqkT", [HID, QK_F], BF16, isOutput=False)
    wvT = nc.declare_dram_parameter("wvT", [HID, HD], BF16, isOutput=False)
    woT = nc.declare_dram_parameter("woT", [Q_SIZE, HOUT], BF16, isOutput=False)
    cosT = nc.declare_dram_parameter("cosT", [128, S], F32, isOutput=False)
    sinT = nc.declare_dram_parameter("sinT", [128, S], F32, isOutput=False)
    masks = nc.declare_dram_parameter("masks", [8, 128, 512], BF16, isOutput=False)
    out = nc.declare_dram_parameter("out", [S, HOUT], F32, isOutput=True)

    hidT_r = hidT.rearrange("(ko p) t -> p ko t", p=128)
    wqkT_r = wqkT.rearrange("(ko p) f -> p ko f", p=128)
    wvT_r = wvT.rearrange("(ko p) d -> p ko d", p=128)
    woT_r = woT.rearrange("(fo p) h -> p fo h", p=128)
    masks_r = masks.rearrange("m p q -> p m q")

    with tile.TileContext(nc) as tc:
        with (
            tc.tile_pool(name="persist", bufs=1) as persist,
            tc.tile_pool(name="dram", bufs=1, space="DRAM") as dram,
        ):
            ag_ins = [dram.tile([2 * HD, S // 4], BF16, name=f"agi{k}")
                      for k in range(4)]
            ag_outs = [dram.tile([Q_SIZE, S // 4], BF16, name=f"ago{k}",
                                 addr_space="Shared") for k in range(4)]

            # live across phases A+B
            qk_sb = persist.tile([128, 6, S], BF16)   # roped qT/kT rows: [h0x1,h0x2,h1x1,h1x2,kx1,kx2]
            v_sb = persist.tile([128, S // 128, HD], BF16)  # v in [token, d] layout
            ones_sb = persist.tile([128, 1], BF16)
            nc.vector.memset(ones_sb, 1.0)

            _phase_a(nc, tc, qk_sb, v_sb, hidT_r, wqkT_r, wvT_r, cosT, sinT)

            with tc.tile_pool(name="wo", bufs=1) as wo_pool:
                # prefetch o-proj weights during attention
                wo_sb = wo_pool.tile([128, Q_SIZE // 128, HOUT], BF16)
                nc.sync.dma_start(wo_sb, woT_r)

                _phase_b(nc, tc, qk_sb, v_sb, ones_sb, ag_ins, ag_outs, masks_r)
                _phase_c(nc, tc, wo_sb, ag_outs, out)

    nc.compile()
    return nc


def get_nc():
    if "nc" not in _NC_CACHE:
        _NC_CACHE["nc"] = build_nc()
    return _NC_CACHE["nc"]


def prep_in_maps(inputs):
    bf16 = ml_dtypes.bfloat16
    hs = np.asarray(inputs["hidden_states"], dtype=np.float32)
    pos = np.asarray(inputs["position_ids"]).reshape(-1).astype(np.float64)
    w_qkv = np.asarray(inputs["w_qkv"], dtype=np.float32)
    w_o = np.asarray(inputs["w_o"], dtype=np.float32)

    hidT = np.ascontiguousarray(hs.reshape(S, HID).T).astype(bf16)

    inv_freq = 1.0 / (THETA ** (np.arange(HD // 2, dtype=np.float64) * 2.0 / HD))
    ang = inv_freq[:, None] * pos[None, :]
    cosT = np.cos(ang).astype(np.float32)
    sinT = np.sin(ang).astype(np.float32)

    kk = np.arange(128)[:, None]
    qq = np.arange(512)[None, :]
    masks = np.stack(
        [((qq - kk + o >= 0) & (qq - kk + o <= WINDOW)) for o in MASK_OFFS]
    ).astype(bf16)

    in_maps = []
    for c in range(N_CORES):
        wq = w_qkv[512 * c:512 * (c + 1)]
        wk = w_qkv[Q_SIZE + HD * c:Q_SIZE + HD * (c + 1)]
        wv = w_qkv[Q_SIZE + NKV * HD + HD * c:Q_SIZE + NKV * HD + HD * (c + 1)]
        wqkT = np.ascontiguousarray(np.concatenate([wq, wk], 0).T).astype(bf16)
        wvT = np.ascontiguousarray(wv.T).astype(bf16)
        woT = np.ascontiguousarray(w_o[HOUT * c:HOUT * (c + 1)].T).astype(bf16)
        in_maps.append(
            dict(hidT=hidT, wqkT=wqkT, wvT=wvT, woT=woT,
                 cosT=cosT, sinT=sinT, masks=masks)
        )
    return in_maps


def run(inputs, **kwargs):
    nc = get_nc()
    in_maps = prep_in_maps(inputs)
    return run_bass_kernel_spmd(nc, in_maps, list(range(N_CORES)), **kwargs)


def assemble(res):
    outs = [res.results[c]["out"] for c in range(N_CORES)]
    full = np.concatenate(outs, axis=1).astype(np.float32)
    return full.reshape(1, S, HID)


def kernel(**inputs):
    return assemble(run(inputs))

